# revision 1
# baseline (speedup 1.0000x reference)
"""Trainium2 Bass kernel for a GNN message-passing layer.

Reference computation (per node n, neighbors k=0..31):
  sa = src_atom_emb[atomic]            [N,128]
  ta = tgt_atom_emb[atomic]            [N,128]
  sd = silu(nde @ src_dir_W + b)       [N,64]
  td = silu(nde @ tgt_dir_W + b)       [N,64]
  edist = silu(ede @ dist_W + b)       [N,K,128]
  feat  = [edist | sd[nbr] | sa[nbr] | td | ta]   [N,K,512]
  out   = sum_k(mask*feat) / (sum_k mask + 1e-5)  [N,512]

Strategy (8 cores, nodes sharded 1250/core, SPMD, no collectives):
  - Each core redundantly builds the full per-node feature table
    T[m] = [sd[m] | sa[m]] (10112 rows + zero row) in its DRAM.  sd via
    PE matmul + SiLU; sa via a matmul of a host-encoded one-hot of the
    atomic numbers (bf16, exact) against the bf16 atom embedding,
    which avoids per-row gather descriptors.
  - dist branch: host pre-transposes+pre-masks ede to [128c, E] so the
    PE streams it against dist_W; ACT applies SiLU; DVE does the
    grouped (32-wide) free-axis reduction.  mask*silu(x) ==
    silu(mask*x) for 0/1 masks since silu(0)=0.
  - neighbor gathers: the SWDGE gather costs ~8ns per row regardless
    of row size, so the host compacts masked-out edges away into two
    tiers per 128-node group: K1=14 fixed slots per node (reduced by a
    single strided DVE reduce) plus C2=4 overflow chunks for nodes
    with >14 valid neighbors (reduced by host-encoded 0/1 selection
    matmuls accumulating in PSUM; binomial margins are ~8 sigma).  One
    dma_gather per group pulls all 18 chunks; the dist branch's first
    loads are gated behind the table write so its fp32 matmuls run
    inside the gather window instead of delaying it.
  - recv segments: td/ta for local nodes scaled by cnt/(cnt+1e-5).
"""

import os
import sys

import numpy as np

sys.path.insert(0, "/opt/trn_rl_repo")

import concourse.bacc as bacc  # noqa: E402
import concourse.bass as bass  # noqa: E402,F401
import concourse.mybir as mybir  # noqa: E402
import concourse.tile as tile  # noqa: E402
from concourse.bass_utils import run_bass_kernel_spmd  # noqa: E402

# Problem shape (hardcoded; harness always uses these).
N_CORES = 8
N = 10000
K = 32
NLOC = N // N_CORES          # 1250 nodes per core
NPAD = 1280                  # padded to 10 groups of 128
NG = NPAD // 128             # 10 node groups
E = NPAD * K                 # 40960 edge slots (dist branch layout)
ET = E // 512                # 80 dist tiles of 512 edges
D_DIR_IN = 10
D_DIR = 64
D_ATOM = 128
D_DIST_IN = 128
D_DIST = 128
NUM_ELEM = 100
TROWS = 10112                # 79*128 table build rows (N padded)
ZROW = TROWS                 # zero row index for pad slots
TBCH = TROWS // 128          # 79 table build chunks
DSUM = D_DIR + D_ATOM        # 192 table row width
K1 = 14                      # tier-1 fixed neighbor slots per node
C2 = 4                       # tier-2 overflow chunks per group
CCH = K1 + C2                # gathered chunks per node group
CSLOT = 128 * CCH            # 2560 gather slots per group
OWNER_PAD = 999.0
FP32 = mybir.dt.float32
BF16 = mybir.dt.bfloat16
I16 = mybir.dt.int16

_CACHED = {}
KVAR = os.environ.get("KVAR", "full")


def _build_program():
    nc = bacc.Bacc(
        "TRN2",
        target_bir_lowering=False,
        debug=False,
        enable_asserts=False,
        num_devices=N_CORES,
    )

    edeT = nc.dram_tensor("edeT", [128, E], FP32, kind="ExternalInput")
    nbr_idx = nc.dram_tensor(
        "nbr_idx", [128, NG * CSLOT // 16], I16, kind="ExternalInput"
    )
    oh_all = nc.dram_tensor("oh_all", [128, TROWS], BF16, kind="ExternalInput")
    oh_loc = nc.dram_tensor("oh_loc", [128, NPAD], BF16, kind="ExternalInput")
    sel_h = nc.dram_tensor(
        "sel_h", [128, NG * C2 * 128], FP32, kind="ExternalInput"
    )
    ndeTf = nc.dram_tensor("ndeTf", [D_DIR_IN + 1, TROWS], FP32, kind="ExternalInput")
    ndeTl = nc.dram_tensor("ndeTl", [D_DIR_IN + 1, NPAD], FP32, kind="ExternalInput")
    maskf = nc.dram_tensor("maskf", [128, NG * K], FP32, kind="ExternalInput")
    w_dist = nc.dram_tensor("w_dist", [D_DIST_IN, D_DIST], FP32, kind="ExternalInput")
    w_sd = nc.dram_tensor("w_sd", [D_DIR_IN + 1, D_DIR], FP32, kind="ExternalInput")
    w_td = nc.dram_tensor("w_td", [D_DIR_IN + 1, D_DIR], FP32, kind="ExternalInput")
    emb_s = nc.dram_tensor("emb_s", [128, D_ATOM], BF16, kind="ExternalInput")
    emb_t = nc.dram_tensor("emb_t", [128, D_ATOM], BF16, kind="ExternalInput")
    ident = nc.dram_tensor("ident", [128, 128], FP32, kind="ExternalInput")

    out_d = nc.dram_tensor("out", [NLOC, 512], FP32, kind="ExternalOutput")
    table = nc.dram_tensor("table", [TROWS + 1, DSUM], FP32)

    Silu = mybir.ActivationFunctionType.Silu
    Add = mybir.AluOpType.add
    IsEq = mybir.AluOpType.is_equal
    X = mybir.AxisListType.X

    with tile.TileContext(nc) as tc:
        from contextlib import ExitStack

        with ExitStack() as ctx:
            const = ctx.enter_context(tc.tile_pool(name="const", bufs=1))
            psum_sm = ctx.enter_context(
                tc.tile_pool(name="psum_sm", bufs=2, space="PSUM")
            )
            psum_big = ctx.enter_context(
                tc.tile_pool(name="psum_big", bufs=3, space="PSUM")
            )
            psum_sum = ctx.enter_context(
                tc.tile_pool(name="psum_sum", bufs=2, space="PSUM")
            )
            ede_pool = ctx.enter_context(tc.tile_pool(name="ede_pool", bufs=4))
            silu_pool = ctx.enter_context(tc.tile_pool(name="silu_pool", bufs=3))
            gat_pool = ctx.enter_context(tc.tile_pool(name="gat_pool", bufs=4))
            sel_pool = ctx.enter_context(tc.tile_pool(name="sel_pool", bufs=3))
            out_pool = ctx.enter_context(tc.tile_pool(name="out_pool", bufs=2))
            acc_pool = ctx.enter_context(tc.tile_pool(name="acc_pool", bufs=1))

            # --- constants into SBUF ---
            w_dist_s = const.tile([D_DIST_IN, D_DIST], FP32)
            nc.sync.dma_start(w_dist_s[:], w_dist[:, :])
            w_sd_s = const.tile([D_DIR_IN + 1, D_DIR], FP32)
            nc.sync.dma_start(w_sd_s[:], w_sd[:, :])
            w_td_s = const.tile([D_DIR_IN + 1, D_DIR], FP32)
            nc.sync.dma_start(w_td_s[:], w_td[:, :])
            ident_s = const.tile([128, 128], FP32)
            nc.sync.dma_start(ident_s[:], ident[:, :])
            emb_s_s = const.tile([128, D_ATOM], BF16)
            nc.sync.dma_start(emb_s_s[:], emb_s[:, :])
            ndeTl_s = const.tile([D_DIR_IN + 1, NPAD], FP32)
            nc.sync.dma_start(ndeTl_s[:], ndeTl[:, :])
            maskf_s = const.tile([128, NG * K], FP32)
            nc.sync.dma_start(maskf_s[:], maskf[:, :])
            nbr_idx_s = const.tile([128, NG * CSLOT // 16], I16)
            nc.sync.dma_start(nbr_idx_s[:], nbr_idx[:, :])

            zrow = const.tile([1, DSUM], FP32)
            nc.vector.memset(zrow[:], 0.0)
            nc.sync.dma_start(table[ZROW : ZROW + 1, :], zrow[:])

            # --- P0: build the [sd | sa] node feature table ---
            table_writes = []
            with ExitStack() as p0:
                build_pool = p0.enter_context(tc.tile_pool(name="build_pool", bufs=1))
                ndeTf_s = build_pool.tile([D_DIR_IN + 1, TROWS], FP32)
                nc.sync.dma_start(ndeTf_s[:], ndeTf[:, :])
                oh_all_s = build_pool.tile([128, TROWS], BF16)
                nc.sync.dma_start(oh_all_s[:], oh_all[:, :])

                for half, (b0, b1) in enumerate([(0, 27), (27, 54), (54, TBCH)]):
                  comb = build_pool.tile([128, 27, DSUM], FP32, tag="comb")
                  for b in range(b0, b1):
                        ps_sd = psum_sm.tile([128, 128], FP32, tag="ps_small")
                        nc.tensor.matmul(
                            ps_sd[:, :D_DIR],
                            ndeTf_s[:, b * 128 : (b + 1) * 128],
                            w_sd_s[:],
                            start=True,
                            stop=True,
                        )
                        nc.scalar.activation(
                            comb[:, b - b0, :D_DIR], ps_sd[:, :D_DIR], Silu
                        )
                        ps_sa = psum_sm.tile([128, 128], FP32, tag="ps_small")
                        nc.tensor.matmul(
                            ps_sa[:],
                            oh_all_s[:, b * 128 : (b + 1) * 128],
                            emb_s_s[:],
                            start=True,
                            stop=True,
                        )
                        nc.vector.tensor_copy(comb[:, b - b0, D_DIR:DSUM], ps_sa[:])
                  tview = table[b0 * 128 : b1 * 128, :].rearrange(
                      "(c p) d -> p c d", p=128
                  )
                  table_writes.append(
                      nc.sync.dma_start(tview, comb[:, : b1 - b0, :]).ins
                  )

            # --- P1: dist branch ---
            dist_acc = acc_pool.tile([128, NPAD], FP32)
            from concourse.tile import add_dep_helper

            for j in range(ET):
                t_ede = ede_pool.tile([128, 512], FP32)
                ld = nc.sync.dma_start(t_ede[:], edeT[:, j * 512 : (j + 1) * 512])
                if j < 4:
                    add_dep_helper(
                        ld.ins, table_writes[0], reason="hold dist until table h1"
                    )
                ps_d = psum_big.tile([128, 512], FP32)
                nc.tensor.matmul(ps_d[:], w_dist_s[:], t_ede[:], start=True, stop=True)
                t_silu = silu_pool.tile([128, 512], FP32)
                nc.scalar.activation(t_silu[:], ps_d[:], Silu)
                nc.vector.tensor_reduce(
                    dist_acc[:, j * 16 : (j + 1) * 16],
                    t_silu[:].rearrange("p (n k) -> p n k", k=K),
                    X,
                    Add,
                )

            # --- P2: local node stats / recv features ---
            emb_t_s = const.tile([128, D_ATOM], BF16)
            nc.sync.dma_start(emb_t_s[:], emb_t[:, :])
            oh_loc_s = const.tile([128, NPAD], BF16)
            nc.sync.dma_start(oh_loc_s[:], oh_loc[:, :])
            sel_s = const.tile([128, NG * C2 * 128], FP32)
            nc.sync.dma_start(sel_s[:], sel_h[:, :])
            ta_loc = acc_pool.tile([128, NG, D_ATOM], FP32)
            td_loc = acc_pool.tile([128, NG, D_DIR], FP32)
            for g in range(NG):
                ps_ta = psum_sm.tile([128, 128], FP32, tag="ps_small")
                nc.tensor.matmul(
                    ps_ta[:],
                    oh_loc_s[:, g * 128 : (g + 1) * 128],
                    emb_t_s[:],
                    start=True,
                    stop=True,
                )
                nc.vector.tensor_copy(ta_loc[:, g, :], ps_ta[:])
                ps_td = psum_sm.tile([128, 128], FP32, tag="ps_small")
                nc.tensor.matmul(
                    ps_td[:, :D_DIR],
                    ndeTl_s[:, g * 128 : (g + 1) * 128],
                    w_td_s[:],
                    start=True,
                    stop=True,
                )
                nc.scalar.activation(td_loc[:, g, :], ps_td[:, :D_DIR], Silu)
            cnt = acc_pool.tile([128, NG], FP32)
            nc.vector.tensor_reduce(
                cnt[:], maskf_s[:].rearrange("p (g k) -> p g k", k=K), X, Add
            )
            cnte = acc_pool.tile([128, NG], FP32)
            nc.vector.tensor_scalar_add(cnte[:], cnt[:], 1e-5)
            inv = acc_pool.tile([128, NG], FP32)
            nc.vector.reciprocal(inv[:], cnte[:])
            cim = acc_pool.tile([128, NG], FP32)
            nc.vector.tensor_mul(cim[:], cnt[:], inv[:])

            # --- P3: compacted gather + selection-matmul reduce + output ---
            for g in range(NG):
                gat = gat_pool.tile([128, CCH, DSUM], FP32)
                nc.gpsimd.dma_gather(
                    gat[:],
                    table[:, :],
                    nbr_idx_s[:, g * (CSLOT // 16) : (g + 1) * (CSLOT // 16)],
                    CSLOT,
                    CSLOT,
                    DSUM,
                    single_packet=False,
                )
                t1 = out_pool.tile([128, DSUM], FP32, tag="t1")
                nc.vector.tensor_reduce(
                    t1[:],
                    gat[:, :K1, :].rearrange("p k d -> p d k"),
                    X,
                    Add,
                )
                ps_sum = psum_sum.tile([128, DSUM], FP32)
                for c in range(C2):
                    nc.tensor.matmul(
                        ps_sum[:],
                        sel_s[:, (g * C2 + c) * 128 : (g * C2 + c + 1) * 128],
                        gat[:, K1 + c, :],
                        start=(c == 0),
                        stop=(c == C2 - 1),
                    )
                tsum = out_pool.tile([128, DSUM], FP32, tag="tsum")
                nc.vector.tensor_add(tsum[:], t1[:], ps_sum[:])
                ps_tr = psum_big.tile([128, 128], FP32, tag="ps_tr", bufs=1)
                nc.tensor.transpose(
                    ps_tr[:], dist_acc[:, g * 128 : (g + 1) * 128], ident_s[:]
                )
                out_t = out_pool.tile([128, 512], FP32)
                nc.scalar.mul(out_t[:, 0:128], ps_tr[:], inv[:, g : g + 1])
                nc.scalar.mul(out_t[:, 128:320], tsum[:], inv[:, g : g + 1])
                nc.vector.tensor_scalar_mul(
                    out_t[:, 320:384], td_loc[:, g, :], cim[:, g : g + 1]
                )
                nc.vector.tensor_scalar_mul(
                    out_t[:, 384:512], ta_loc[:, g, :], cim[:, g : g + 1]
                )
                rows = min(128, NLOC - g * 128)
                nc.sync.dma_start(
                    out_d[g * 128 : g * 128 + rows, :], out_t[:rows, :]
                )

    nc.compile()
    return nc


def _wrap_idx(idxs):
    """[M] ints -> [128, M/16] int16 in the dma_gather wrapped layout."""
    m = idxs.shape[0]
    assert m % 16 == 0
    w = np.ascontiguousarray(idxs.astype(np.int16).reshape(m // 16, 16).T)
    return np.ascontiguousarray(np.tile(w, (8, 1)))


def _prep_core(c, atomic, nde, ede, nbr, mask):
    f32 = np.float32
    lo, hi = c * NLOC, (c + 1) * NLOC
    a_loc = atomic[lo:hi]
    nde_loc = nde[lo:hi]
    ede_loc = ede[lo:hi]
    nbr_loc = nbr[lo:hi]
    mask_loc = mask[lo:hi]

    # dist branch input: premasked, transposed, padded to E columns.
    em = (ede_loc * mask_loc[:, :, None].astype(f32)).reshape(NLOC * K, D_DIST_IN)
    edeT = np.zeros((128, E), dtype=f32)
    edeT[:, : NLOC * K] = em.T
    edeT = np.ascontiguousarray(edeT)

    # two-tier compacted gather: tier-1 = first K1 valid neighbors per
    # node at fixed slots [k, p]; tier-2 = overflow edges packed into C2
    # chunks per group with an owner (node-within-group) map.
    idx_all = np.full((NG, CCH, 128), ZROW, dtype=np.int32)
    own_t2 = np.full((NG, C2, 128), OWNER_PAD, dtype=f32)
    mn = np.full((NPAD, K), -1, dtype=np.int32)
    mn[:NLOC] = np.where(mask_loc, nbr_loc, -1)
    for g in range(NG):
        blk = mn[g * 128 : (g + 1) * 128]
        ov_rows = []
        ov_nodes = []
        for p in range(128):
            valid = blk[p][blk[p] >= 0]
            n1 = min(K1, valid.shape[0])
            idx_all[g, :n1, p] = valid[:n1]
            if valid.shape[0] > K1:
                ov_rows.append(valid[K1:])
                ov_nodes.append(np.full(valid.shape[0] - K1, p))
        if ov_rows:
            ov_rows = np.concatenate(ov_rows)
            ov_nodes = np.concatenate(ov_nodes)
            v = ov_rows.shape[0]
            assert v <= C2 * 128, f"group {g} overflow {v} > {C2 * 128}"
            flat_idx = idx_all[g, K1:].reshape(-1)
            flat_idx[:v] = ov_rows
            idx_all[g, K1:] = flat_idx.reshape(C2, 128)
            flat_own = own_t2[g].reshape(-1)
            flat_own[:v] = ov_nodes.astype(f32)
            own_t2[g] = flat_own.reshape(C2, 128)
    nbr_idx = _wrap_idx(idx_all.reshape(-1))
    # selection matrices sel[p, (g,c), n] = 1 if own_t2[g, c, p] == n
    sel_m = np.zeros((NG, C2, 128, 128), dtype=f32)
    gg, cc, pp = np.nonzero(own_t2 != OWNER_PAD)
    sel_m[gg, cc, pp, own_t2[gg, cc, pp].astype(np.int64)] = 1.0
    sel_h = np.ascontiguousarray(
        sel_m.transpose(2, 0, 1, 3).reshape(128, NG * C2 * 128)
    )

    import ml_dtypes
    oh_loc = np.zeros((128, NPAD), dtype=ml_dtypes.bfloat16)
    cols = np.arange(NLOC)
    oh_loc[a_loc.astype(np.int64), cols] = 1.0

    ndeTl = np.zeros((D_DIR_IN + 1, NPAD), dtype=f32)
    ndeTl[:D_DIR_IN, :NLOC] = nde_loc.T
    ndeTl[D_DIR_IN, :] = 1.0

    mpad = np.zeros((NPAD, K), dtype=f32)
    mpad[:NLOC] = mask_loc.astype(f32)
    maskf = np.ascontiguousarray(
        mpad.reshape(NG, 128, K).transpose(1, 0, 2).reshape(128, NG * K)
    )

    return {
        "edeT": edeT,
        "nbr_idx": nbr_idx,
        "sel_h": sel_h,
        "oh_loc": np.ascontiguousarray(oh_loc),
        "ndeTl": np.ascontiguousarray(ndeTl),
        "maskf": maskf,
    }


def _prepare_all(inputs):
    f32 = np.float32
    atomic = np.asarray(inputs["atomic_numbers"]).astype(np.int32)
    nde = np.asarray(inputs["node_direction_expansion"]).astype(f32)
    ede = np.asarray(inputs["edge_distance_expansion"]).astype(f32)
    nbr = np.asarray(inputs["neighbor_list"]).astype(np.int32)
    mask = np.asarray(inputs["neighbor_mask"]).astype(bool)
    emb_s = np.asarray(inputs["src_atom_emb"]).astype(f32)
    emb_t = np.asarray(inputs["tgt_atom_emb"]).astype(f32)
    w_sd = np.asarray(inputs["src_dir_W"]).astype(f32)
    b_sd = np.asarray(inputs["src_dir_b"]).astype(f32)
    w_td = np.asarray(inputs["tgt_dir_W"]).astype(f32)
    b_td = np.asarray(inputs["tgt_dir_b"]).astype(f32)
    w_di = np.ascontiguousarray(np.asarray(inputs["dist_W"]).astype(f32))
    b_di = np.asarray(inputs["dist_b"]).astype(f32)
    assert np.all(b_di == 0.0), "nonzero dist_b not supported"

    import ml_dtypes
    oh_all = np.zeros((128, TROWS), dtype=ml_dtypes.bfloat16)
    oh_all[atomic.astype(np.int64), np.arange(N)] = 1.0
    ndeTf = np.zeros((D_DIR_IN + 1, TROWS), dtype=f32)
    ndeTf[:D_DIR_IN, :N] = nde.T
    ndeTf[D_DIR_IN, :] = 1.0
    ndeTf = np.ascontiguousarray(ndeTf)
    emb_s_pad = np.zeros((128, D_ATOM), dtype=f32)
    emb_s_pad[:NUM_ELEM] = emb_s
    emb_t_pad = np.zeros((128, D_ATOM), dtype=f32)
    emb_t_pad[:NUM_ELEM] = emb_t

    shared = {
        "oh_all": np.ascontiguousarray(oh_all),
        "ndeTf": ndeTf,
        "w_dist": w_di,
        "w_sd": np.ascontiguousarray(np.vstack([w_sd, b_sd[None, :]])),
        "w_td": np.ascontiguousarray(np.vstack([w_td, b_td[None, :]])),
        "emb_s": emb_s_pad.astype(ml_dtypes.bfloat16),
        "emb_t": emb_t_pad.astype(ml_dtypes.bfloat16),
        "ident": np.ascontiguousarray(np.eye(128, dtype=f32)),

    }

    in_maps = []
    for c in range(N_CORES):
        m = _prep_core(c, atomic, nde, ede, nbr, mask)
        m.update(shared)
        in_maps.append(m)
    return in_maps


def _run(inputs, trace=False, **spmd_kwargs):
    key = "prog"
    if key not in _CACHED:
        _CACHED[key] = _build_program()
    nc = _CACHED[key]

    in_maps = _prepare_all(inputs)
    res = run_bass_kernel_spmd(
        nc, in_maps, list(range(N_CORES)), trace=trace, **spmd_kwargs
    )
    out = np.concatenate([res.results[c]["out"] for c in range(N_CORES)], axis=0)
    return out.astype(np.float32), res


def kernel(**inputs):
    out, _ = _run(inputs, trace=False)
    return out



# revision 5
# speedup vs baseline: 4.3609x; 4.3609x over previous
"""Trainium2 Bass kernel for a GNN message-passing layer.

Reference computation (per node n, neighbors k=0..31):
  sa = src_atom_emb[atomic]            [N,128]
  ta = tgt_atom_emb[atomic]            [N,128]
  sd = silu(nde @ src_dir_W + b)       [N,64]
  td = silu(nde @ tgt_dir_W + b)       [N,64]
  edist = silu(ede @ dist_W + b)       [N,K,128]
  feat  = [edist | sd[nbr] | sa[nbr] | td | ta]   [N,K,512]
  out   = sum_k(mask*feat) / (sum_k mask + 1e-5)  [N,512]

Strategy (8 cores, nodes sharded 1250/core, SPMD, no collectives, and
NO on-device gather):

  - The host drops masked-out edges entirely and packs the ~2000 valid
    edges of each 128-node group into CH=17 chunks of 128 edge slots,
    sorted by receiver.  Per-edge streams:
      edeC [128, slots] fp32   edge_distance_expansion, feature-major
      ndeE [11, slots]  fp16   SOURCE node's direction expansion + ones
      selN [128, chunks] fp32  receiver node id of each slot (999 pad)
  - Per chunk the PE computes edge-major z = ede@W (fp32 2-pass) and
    z_sd = nde_src@W_sd (fp16); ACT applies SiLU into fp16 tiles; DVE
    expands selN into a 0/1 staircase via iota==selN (exact in fp16);
    one selection matmul per chunk (fp16, 1 cyc/row) accumulates the
    per-receiver [dist|sd] sums in PSUM.
  - Sender-atom sums collapse via a host-built histogram over the 100
    elements (integer counts, exact in fp16): one matmul per group.
  - All five output segments accumulate in ONE psum bank per group
    ([dist|sd|sa|td|ta] = 512 fp32), then DVE scales by 1/(cnt+1e-5)
    (and cnt/(cnt+1e-5) for the receiver segments).
  - Precision: ede/dist_W stay fp32 (bf16/fp16 quantization of the
    dist matmul would breach the 2e-2 scale-relative gate); the tiny
    nde/sel/emb streams are fp16 or exact.
  - DMA issue is split: Sync streams edeC/ndeE/output, the otherwise
    idle GpSimd engine issues constants, so group 0 starts immediately.
"""

import numpy as np
import sys

sys.path.insert(0, "/opt/trn_rl_repo")

import concourse.bacc as bacc  # noqa: E402
import concourse.bass as bass  # noqa: E402,F401
import concourse.mybir as mybir  # noqa: E402
import concourse.tile as tile  # noqa: E402
from concourse.bass_utils import run_bass_kernel_spmd  # noqa: E402

# Problem shape (hardcoded; harness always uses these).
N_CORES = 8
N = 10000
K = 32
NLOC = N // N_CORES          # 1250 nodes per core
NPAD = 1280                  # padded to 10 groups of 128
NG = NPAD // 128             # 10 node groups
CH = 17                      # edge chunks of 128 per group (max v_g=2119)
SLOTS = CH * 128             # 2176 edge slots per group
D_DIR_IN = 10
D_DIR = 64
D_ATOM = 128
D_DIST_IN = 128
D_DIST = 128
NUM_ELEM = 100
PAD_NODE = 999.0
FP32 = mybir.dt.float32
FP16 = mybir.dt.float16

_CACHED = {}


def _build_program():
    nc = bacc.Bacc(
        "TRN2",
        target_bir_lowering=False,
        debug=False,
        enable_asserts=False,
        num_devices=N_CORES,
    )

    edeC = nc.dram_tensor("edeC", [128, NG * SLOTS], FP32, kind="ExternalInput")
    ndeE = nc.dram_tensor("ndeE", [D_DIR_IN + 1, NG * SLOTS], FP16, kind="ExternalInput")
    selN = nc.dram_tensor("selN", [128, NG * CH], FP32, kind="ExternalInput")
    iota = nc.dram_tensor("iota", [128, 128], FP16, kind="ExternalInput")
    hsT = nc.dram_tensor("hsT", [128, NPAD], FP16, kind="ExternalInput")
    ohT = nc.dram_tensor("ohT", [128, NPAD], FP16, kind="ExternalInput")
    ndeLT = nc.dram_tensor("ndeLT", [D_DIR_IN + 1, NPAD], FP32, kind="ExternalInput")
    cntf = nc.dram_tensor("cntf", [128, NG], FP32, kind="ExternalInput")
    w_dist = nc.dram_tensor("w_dist", [D_DIST_IN, D_DIST], FP32, kind="ExternalInput")
    w_sd = nc.dram_tensor("w_sd", [D_DIR_IN + 1, D_DIR], FP16, kind="ExternalInput")
    w_td = nc.dram_tensor("w_td", [D_DIR_IN + 1, D_DIR], FP32, kind="ExternalInput")
    emb_s = nc.dram_tensor("emb_s", [128, D_ATOM], FP16, kind="ExternalInput")
    emb_t = nc.dram_tensor("emb_t", [128, D_ATOM], FP16, kind="ExternalInput")

    out_d = nc.dram_tensor("out", [NLOC, 512], FP32, kind="ExternalOutput")

    Silu = mybir.ActivationFunctionType.Silu
    IsEq = mybir.AluOpType.is_equal

    with tile.TileContext(nc) as tc:
        from contextlib import ExitStack

        with ExitStack() as ctx:
            const = ctx.enter_context(tc.tile_pool(name="const", bufs=1))
            pD = ctx.enter_context(tc.tile_pool(name="pD", bufs=2, space="PSUM"))
            pS = ctx.enter_context(tc.tile_pool(name="pS", bufs=2, space="PSUM"))
            pA = ctx.enter_context(tc.tile_pool(name="pA", bufs=2, space="PSUM"))
            ede_pool = ctx.enter_context(tc.tile_pool(name="ede_pool", bufs=3))
            nde_pool = ctx.enter_context(tc.tile_pool(name="nde_pool", bufs=3))
            sel_pool = ctx.enter_context(tc.tile_pool(name="sel_pool", bufs=3))
            # agg reads every silu tile of a group at group end, so pools must
            # hold a full group's batches plus pipelining headroom.
            silD_pool = ctx.enter_context(tc.tile_pool(name="silD_pool", bufs=5))
            silS_pool = ctx.enter_context(tc.tile_pool(name="silS_pool", bufs=5))
            out_pool = ctx.enter_context(tc.tile_pool(name="out_pool", bufs=3))

            # constants: small/early ones on Sync, the rest from GpSimd so the
            # Sync queue reaches group 0's streams immediately.
            w_dist_s = const.tile([D_DIST_IN, D_DIST], FP32)
            nc.sync.dma_start(w_dist_s[:], w_dist[:, :])
            w_sd_s = const.tile([D_DIR_IN + 1, D_DIR], FP16)
            nc.sync.dma_start(w_sd_s[:], w_sd[:, :])
            selN_s = const.tile([128, NG * CH], FP32)
            nc.gpsimd.dma_start(selN_s[:], selN[:, :])
            iota_s = const.tile([128, 128], FP16)
            nc.gpsimd.dma_start(iota_s[:], iota[:, :])
            cnt_s = const.tile([128, NG], FP32)
            nc.gpsimd.dma_start(cnt_s[:], cntf[:, :])
            hsT_s = const.tile([128, NPAD], FP16)
            nc.gpsimd.dma_start(hsT_s[:], hsT[:, :])
            ohT_s = const.tile([128, NPAD], FP16)
            nc.gpsimd.dma_start(ohT_s[:], ohT[:, :])
            ndeLT_s = const.tile([D_DIR_IN + 1, NPAD], FP32)
            nc.gpsimd.dma_start(ndeLT_s[:], ndeLT[:, :])
            emb_s_s = const.tile([128, D_ATOM], FP16)
            nc.gpsimd.dma_start(emb_s_s[:], emb_s[:, :])
            emb_t_s = const.tile([128, D_ATOM], FP16)
            nc.gpsimd.dma_start(emb_t_s[:], emb_t[:, :])
            w_td_s = const.tile([D_DIR_IN + 1, D_DIR], FP32)
            nc.gpsimd.dma_start(w_td_s[:], w_td[:, :])

            cnte = const.tile([128, NG], FP32)
            nc.vector.tensor_scalar_add(cnte[:], cnt_s[:], 1e-5)
            inv = const.tile([128, NG], FP32)
            nc.vector.reciprocal(inv[:], cnte[:])
            cim = const.tile([128, NG], FP32)
            nc.vector.tensor_mul(cim[:], cnt_s[:], inv[:])

            NB_D = (CH + 7) // 8   # dist silu batches of 8 chunks (2-bank psum)
            NB_S = (CH + 7) // 8   # sd silu batches of 8 chunks

            for g in range(NG):
                ede_t = ede_pool.tile([128, SLOTS], FP32)
                h1 = 9 * 128
                nc.sync.dma_start(
                    ede_t[:, :h1], edeC[:, g * SLOTS : g * SLOTS + h1]
                )
                nc.sync.dma_start(
                    ede_t[:, h1:], edeC[:, g * SLOTS + h1 : (g + 1) * SLOTS]
                )
                nde_t = nde_pool.tile([D_DIR_IN + 1, SLOTS], FP16)
                nc.sync.dma_start(nde_t[:], ndeE[:, g * SLOTS : (g + 1) * SLOTS])

                # 0/1 staircase selection matrices from iota == selN
                sel_t = sel_pool.tile([128, SLOTS], FP16)
                for c in range(CH):
                    nc.vector.tensor_scalar(
                        sel_t[:, c * 128 : (c + 1) * 128],
                        iota_s[:],
                        selN_s[:, g * CH + c : g * CH + c + 1],
                        None,
                        IsEq,
                    )

                # dist z = ede@W per chunk, silu into fp16 tiles (batch 8)
                silD = []
                for b in range(NB_D):
                    c0, c1 = b * 8, min(b * 8 + 8, CH)
                    w = (c1 - c0) * 128
                    ps = pD.tile([128, 1024], FP32, tag="psD")
                    for c in range(c0, c1):
                        nc.tensor.matmul(
                            ps[:, (c - c0) * 128 : (c - c0 + 1) * 128],
                            ede_t[:, c * 128 : (c + 1) * 128],
                            w_dist_s[:],
                            start=True,
                            stop=True,
                        )
                    st = silD_pool.tile([128, 1024], FP16, tag="silD")
                    nc.scalar.activation(st[:, :w], ps[:, :w], Silu)
                    silD.append(st)

                # sd z = nde_src@W_sd per chunk, silu into fp16 (batch 8)
                silS = []
                for b in range(NB_S):
                    c0, c1 = b * 8, min(b * 8 + 8, CH)
                    w = (c1 - c0) * 64
                    ps = pS.tile([128, 512], FP32, tag="psS")
                    for c in range(c0, c1):
                        nc.tensor.matmul(
                            ps[:, (c - c0) * 64 : (c - c0 + 1) * 64],
                            nde_t[:, c * 128 : (c + 1) * 128],
                            w_sd_s[:],
                            start=True,
                            stop=True,
                        )
                    st = silS_pool.tile([128, 512], FP16, tag="silS")
                    nc.scalar.activation(st[:, :w], ps[:, :w], Silu)
                    silS.append(st)

                # one psum bank accumulates the full 512-wide output row:
                # [dist 0:128 | sd 128:192 | sa 192:320 | td 320:384 | ta 384:512]
                psA = pA.tile([128, 512], FP32, tag="psA")
                for c in range(CH):
                    nc.tensor.matmul(
                        psA[:, 0:128],
                        sel_t[:, c * 128 : (c + 1) * 128],
                        silD[c // 8][:, (c % 8) * 128 : (c % 8 + 1) * 128],
                        start=(c == 0),
                        stop=(c == CH - 1),
                    )
                for c in range(CH):
                    nc.tensor.matmul(
                        psA[:, 128:192],
                        sel_t[:, c * 128 : (c + 1) * 128],
                        silS[c // 8][:, (c % 8) * 64 : (c % 8 + 1) * 64],
                        start=(c == 0),
                        stop=(c == CH - 1),
                    )
                nc.tensor.matmul(
                    psA[:, 192:320],
                    hsT_s[:, g * 128 : (g + 1) * 128],
                    emb_s_s[:],
                    start=True,
                    stop=True,
                )
                nc.tensor.matmul(
                    psA[:, 320:384],
                    ndeLT_s[:, g * 128 : (g + 1) * 128],
                    w_td_s[:],
                    start=True,
                    stop=True,
                )
                nc.tensor.matmul(
                    psA[:, 384:512],
                    ohT_s[:, g * 128 : (g + 1) * 128],
                    emb_t_s[:],
                    start=True,
                    stop=True,
                )

                td_t = out_pool.tile([128, D_DIR], FP32, tag="td")
                nc.scalar.activation(td_t[:], psA[:, 320:384], Silu)

                out_t = out_pool.tile([128, 512], FP32, tag="out")
                nc.vector.tensor_scalar_mul(out_t[:, 0:320], psA[:, 0:320], inv[:, g : g + 1])
                nc.vector.tensor_scalar_mul(
                    out_t[:, 320:384], td_t[:], cim[:, g : g + 1]
                )
                nc.vector.tensor_scalar_mul(
                    out_t[:, 384:512], psA[:, 384:512], cim[:, g : g + 1]
                )
                rows = min(128, NLOC - g * 128)
                nc.sync.dma_start(out_d[g * 128 : g * 128 + rows, :], out_t[:rows, :])

    nc.compile()
    return nc


def _prep_core(c, atomic, nde, ede, nbr, mask):
    f32 = np.float32
    f16 = np.float16
    lo, hi = c * NLOC, (c + 1) * NLOC
    a_loc = atomic[lo:hi]
    nde_loc = nde[lo:hi]
    ede_loc = ede[lo:hi]
    nbr_loc = nbr[lo:hi]
    mask_loc = mask[lo:hi]

    edeC = np.zeros((128, NG * SLOTS), dtype=f32)
    ndeE = np.zeros((D_DIR_IN + 1, NG * SLOTS), dtype=f16)
    selN = np.full((128, NG * CH), PAD_NODE, dtype=f32)
    hs = np.zeros((128, NPAD), dtype=np.int32)
    ohT = np.zeros((128, NPAD), dtype=f16)
    cnt = np.zeros((128, NG), dtype=f32)

    for g in range(NG):
        base = g * 128
        nn = min(128, NLOC - base)
        gm = mask_loc[base : base + nn]                     # [nn, K]
        ni, ki = np.nonzero(gm)                             # receiver-major order
        E = ni.shape[0]
        assert E <= SLOTS, f"group {g} edges {E} > {SLOTS}"
        src = nbr_loc[base + ni, ki]                        # global source ids
        ee = np.arange(E)
        edeC[:, g * SLOTS + ee] = ede_loc[base + ni, ki, :].T
        ndeE[:D_DIR_IN, g * SLOTS + ee] = nde[src].T
        ndeE[D_DIR_IN, g * SLOTS + ee] = 1.0
        selN[ee % 128, g * CH + ee // 128] = ni
        np.add.at(hs, (atomic[src], base + ni), 1)
        ohT[a_loc[base : base + nn], base + np.arange(nn)] = 1.0
        cnt[:nn, g] = gm.sum(1)

    ndeLT = np.zeros((D_DIR_IN + 1, NPAD), dtype=f32)
    ndeLT[:D_DIR_IN, :NLOC] = nde_loc.T
    ndeLT[D_DIR_IN, :] = 1.0

    return {
        "edeC": edeC,
        "ndeE": ndeE,
        "selN": selN,
        "hsT": hs.astype(f16),
        "ohT": ohT,
        "ndeLT": ndeLT,
        "cntf": cnt,
    }


def _prepare_all(inputs):
    f32 = np.float32
    f16 = np.float16
    atomic = np.asarray(inputs["atomic_numbers"]).astype(np.int64)
    nde = np.asarray(inputs["node_direction_expansion"]).astype(f32)
    ede = np.asarray(inputs["edge_distance_expansion"]).astype(f32)
    nbr = np.asarray(inputs["neighbor_list"]).astype(np.int64)
    mask = np.asarray(inputs["neighbor_mask"]).astype(bool)
    emb_s = np.asarray(inputs["src_atom_emb"]).astype(f32)
    emb_t = np.asarray(inputs["tgt_atom_emb"]).astype(f32)
    w_sd = np.asarray(inputs["src_dir_W"]).astype(f32)
    b_sd = np.asarray(inputs["src_dir_b"]).astype(f32)
    w_td = np.asarray(inputs["tgt_dir_W"]).astype(f32)
    b_td = np.asarray(inputs["tgt_dir_b"]).astype(f32)
    w_di = np.ascontiguousarray(np.asarray(inputs["dist_W"]).astype(f32))
    b_di = np.asarray(inputs["dist_b"]).astype(f32)
    assert np.all(b_di == 0.0), "nonzero dist_b not supported"

    emb_s_pad = np.zeros((128, D_ATOM), dtype=f16)
    emb_s_pad[:NUM_ELEM] = emb_s.astype(f16)
    emb_t_pad = np.zeros((128, D_ATOM), dtype=f16)
    emb_t_pad[:NUM_ELEM] = emb_t.astype(f16)

    shared = {
        "w_dist": w_di,
        "w_sd": np.ascontiguousarray(np.vstack([w_sd, b_sd[None, :]]).astype(f16)),
        "w_td": np.ascontiguousarray(np.vstack([w_td, b_td[None, :]])),
        "emb_s": emb_s_pad,
        "emb_t": emb_t_pad,
        "iota": np.ascontiguousarray(
            np.tile(np.arange(128, dtype=f16), (128, 1))
        ),
    }

    in_maps = []
    for c in range(N_CORES):
        m = _prep_core(c, atomic, nde, ede, nbr, mask)
        m.update(shared)
        in_maps.append(m)
    return in_maps


def _run(inputs, trace=False, **spmd_kwargs):
    key = "prog"
    if key not in _CACHED:
        _CACHED[key] = _build_program()
    nc = _CACHED[key]

    in_maps = _prepare_all(inputs)
    res = run_bass_kernel_spmd(
        nc, in_maps, list(range(N_CORES)), trace=trace, **spmd_kwargs
    )
    out = np.concatenate([res.results[c]["out"] for c in range(N_CORES)], axis=0)
    return out.astype(np.float32), res


def kernel(**inputs):
    out, _ = _run(inputs, trace=False)
    return out
